# revision 1
# baseline (speedup 1.0000x reference)
"""GNN message-passing (gather + segment_sum) Trainium2 Bass kernel.

Strategy (node-parallel over destination blocks):
  - Pad node space to 50176 = 8 cores x 49 groups x 128 nodes. Core c owns
    dst nodes [c*6272, (c+1)*6272); no cross-core reduction needed.
  - Host sorts edges by (dst group, src-half) and packs, per core, int16
    gather indices (dma_gather requires int16, so the x table is addressed
    as two halves split at row 32768) plus per-slot group-relative dst
    values (f32, -1 for padding).
  - Device, per 128-node group: dma_gather the edge messages from the
    padded x table (HBM, 256B stride), build a one-hot selection matrix
    B[edge, node] = (dst_rel[edge] == iota[node]) on VectorE, and
    accumulate out_g = sum_chunks B^T @ msgs on TensorE into PSUM (exact
    f32 segment-sum; duplicate dst handled by the matmul reduction).
  - PSUM -> SBUF -> HBM per group; host concatenates core outputs.

Self-contained: hardcodes the problem shapes from the spec.
"""

import math

import numpy as np

import concourse.bass as bass
import concourse.tile as tile
from concourse.bass import _add_dep_helper
from concourse import bacc, mybir
from concourse.alu_op_type import AluOpType
from concourse.bass_utils import run_bass_kernel_spmd

N_NODES = 50000
D_FEAT = 32
N_CORES = 8
G = 128  # dst nodes per group
GROUPS_PER_CORE = 49
N_GROUPS_TOT = N_CORES * GROUPS_PER_CORE  # 392
N_PAD = N_GROUPS_TOT * G  # 50176
LO_ROWS = 32768  # x-table split so gather indices fit int16
ELEM = 64  # f32 per padded x row (256B stride, dma_gather constraint)
CALL = 1024  # max idxs per dma_gather call (SWDGE ring/scratch limit)
MSG_BUFS = 3


def _call_sizes(n_chunks):
    """Split n_chunks*128 idx slots into dma_gather calls of <= CALL idxs."""
    sizes = []
    left = n_chunks * 128
    while left > 0:
        s = min(CALL, left)
        sizes.append(s)
        left -= s
    return sizes


def _prep(x, edge_index):
    """Host-side packing. Returns per-core input maps + schedule constants."""
    src = np.asarray(edge_index[0], dtype=np.int64)
    dst = np.asarray(edge_index[1], dtype=np.int64)
    E = src.shape[0]

    grp = dst >> 7
    half = (src >= LO_ROWS).astype(np.int64)
    order = np.lexsort((half, grp))
    src_s, dst_s, grp_s, half_s = src[order], dst[order], grp[order], half[order]

    key = grp_s * 2 + half_s
    cnt = np.bincount(key, minlength=2 * N_GROUPS_TOT)
    cnt_lo, cnt_hi = cnt[0::2], cnt[1::2]
    L_CH = max(1, math.ceil(int(cnt_lo.max()) / 128))
    H_CH = max(1, math.ceil(int(cnt_hi.max()) / 128))
    C = L_CH + H_CH

    idx_cols = GROUPS_PER_CORE * C * 8  # 16-wrapped idx columns per core
    dr_cols = GROUPS_PER_CORE * C

    slot = np.arange(E) - (np.cumsum(cnt) - cnt)[key]
    core = grp_s // GROUPS_PER_CORE
    g_in = grp_s % GROUPS_PER_CORE

    idx_arr = np.full((N_CORES, 16, idx_cols), -1, np.int16)
    dr_arr = np.full((N_CORES, 128, dr_cols), -1.0, np.float32)

    idxval = (src_s - half_s * LO_ROWS).astype(np.int16)
    icol = g_in * C * 8 + half_s * L_CH * 8 + slot // 16
    idx_arr[core, slot % 16, icol] = idxval
    dcol = g_in * C + half_s * L_CH + slot // 128
    dr_arr[core, slot % 128, dcol] = (dst_s - (grp_s << 7)).astype(np.float32)

    # Per-call valid counts (decode-side ring reservation reads these from a
    # register and they must equal the post-trim index count). Calls whose
    # slots are entirely padding get one guard idx=0 (dst_rel stays -1 ->
    # zero contribution): keeps CoreSim's gather exec and the ucode trim on
    # the nonempty path.
    lo_sizes = _call_sizes(L_CH)
    hi_sizes = _call_sizes(H_CH)
    n_calls = GROUPS_PER_CORE * (len(lo_sizes) + len(hi_sizes))
    cnts = np.zeros((N_CORES, 1, n_calls), np.int32)
    for c in range(N_CORES):
        k = 0
        for g in range(GROUPS_PER_CORE):
            for h, sizes in ((0, lo_sizes), (1, hi_sizes)):
                n_real = int(cnt[(c * GROUPS_PER_CORE + g) * 2 + h])
                off = 0
                for s in sizes:
                    valid = min(max(n_real - off, 0), s)
                    if valid == 0:  # fully-padded call: place guard idx
                        col0 = g * C * 8 + h * L_CH * 8 + off // 16
                        idx_arr[c, 0, col0] = 0
                        valid = 1
                    cnts[c, 0, k] = valid
                    k += 1
                    off += s

    xpad = np.zeros((N_NODES, ELEM), np.float32)
    xpad[:, :D_FEAT] = np.asarray(x, dtype=np.float32)

    iota = np.broadcast_to(
        np.arange(G, dtype=np.float32)[None, :], (128, G)
    ).copy()

    ins = []
    for c in range(N_CORES):
        ins.append(
            {
                "xpad": xpad,
                "idx16": np.tile(idx_arr[c], (8, 1)),
                "dstrel": dr_arr[c],
                "iota": iota,
                "cnts": cnts[c],
            }
        )
    return ins, L_CH, H_CH, idx_cols, dr_cols, n_calls


def _build(reps, L_CH, H_CH, idx_cols, dr_cols, n_calls):
    C = L_CH + H_CH
    nc = bacc.Bacc(
        "TRN2", target_bir_lowering=False, debug=False, num_devices=N_CORES
    )
    f32 = mybir.dt.float32
    xpad = nc.dram_tensor("xpad", [N_NODES, ELEM], f32, kind="ExternalInput")
    idx16 = nc.dram_tensor(
        "idx16", [128, idx_cols], mybir.dt.int16, kind="ExternalInput"
    )
    dstrel = nc.dram_tensor("dstrel", [128, dr_cols], f32, kind="ExternalInput")
    iota = nc.dram_tensor("iota", [128, G], f32, kind="ExternalInput")
    cnts = nc.dram_tensor("cnts", [1, n_calls], mybir.dt.int32, kind="ExternalInput")
    out = nc.dram_tensor(
        "out", [GROUPS_PER_CORE * G, D_FEAT], f32, kind="ExternalOutput"
    )

    lo_sizes = _call_sizes(L_CH)
    hi_sizes = _call_sizes(H_CH)
    x_lo = xpad.ap()[0:LO_ROWS, :]
    x_hi = xpad.ap()[LO_ROWS:N_NODES, :]

    with tile.TileContext(nc) as tc:
        with (
            tc.tile_pool(name="meta", bufs=1) as meta,
            tc.tile_pool(name="msg", bufs=MSG_BUFS) as msgp,
            tc.tile_pool(name="bsel", bufs=4) as bselp,
            tc.tile_pool(name="ps", bufs=2, space="PSUM") as psp,
            tc.tile_pool(name="stage", bufs=2) as stagep,
        ):
            idx_t = meta.tile([128, idx_cols], mybir.dt.int16)
            nc.sync.dma_start(idx_t[:], idx16.ap())
            dr_t = meta.tile([128, dr_cols], f32)
            nc.sync.dma_start(dr_t[:], dstrel.ap())
            iota_t = meta.tile([128, G], f32)
            nc.sync.dma_start(iota_t[:], iota.ap())
            cnt_t = meta.tile([1, n_calls], mybir.dt.int32)
            nc.sync.dma_start(cnt_t[:], cnts.ap())
            cnt_reg = nc.gpsimd.alloc_register("gather_cnt")

            def body(_=None):
                call_k = 0
                prev_gather = None
                for g in range(GROUPS_PER_CORE):
                    msgs = msgp.tile([128, C, ELEM], f32)
                    if g < MSG_BUFS:
                        # virgin SBUF may hold NaN bit patterns; padded slots
                        # must be finite (B row is 0 but 0*NaN = NaN in PE)
                        nc.vector.memset(msgs[:], 0.0)
                    ccol = 0
                    for h, sizes, base_ap in (
                        (0, lo_sizes, x_lo),
                        (1, hi_sizes, x_hi),
                    ):
                        icol = g * C * 8 + h * L_CH * 8
                        for s in sizes:
                            k = s // 128
                            ld = nc.gpsimd.reg_load(
                                cnt_reg, cnt_t[0:1, call_k : call_k + 1]
                            )
                            if prev_gather is not None:
                                # the register is re-used: its load must not
                                # overtake the previous gather's decode
                                _add_dep_helper(
                                    ld.ins, prev_gather.ins, False,
                                    reason="cnt reg WAR on prior gather",
                                )
                            gth = nc.gpsimd.dma_gather(
                                msgs[:, ccol : ccol + k, :],
                                base_ap,
                                idx_t[:, icol : icol + s // 16],
                                s,
                                cnt_reg,
                                ELEM,
                                elem_step=ELEM,
                            )
                            _add_dep_helper(
                                gth.ins, ld.ins, False,
                                reason="num_idxs_reg load before gather",
                            )
                            prev_gather = gth
                            call_k += 1
                            ccol += k
                            icol += s // 16
                    ps = psp.tile([128, D_FEAT], f32)
                    for c in range(C):
                        bt = bselp.tile([128, G], f32)
                        nc.vector.tensor_scalar(
                            bt[:],
                            iota_t[:],
                            dr_t[:, g * C + c : g * C + c + 1],
                            None,
                            AluOpType.is_equal,
                        )
                        nc.tensor.matmul(
                            out=ps[:],
                            lhsT=bt[:],
                            rhs=msgs[:, c, 0:D_FEAT],
                            start=(c == 0),
                            stop=(c == C - 1),
                        )
                    st = stagep.tile([128, D_FEAT], f32)
                    nc.scalar.copy(st[:], ps[:])
                    nc.sync.dma_start(out.ap()[g * G : (g + 1) * G, :], st[:])

            if reps == 1:
                body()
            else:
                with tc.For_i(0, reps) as _i:
                    body(_i)
    nc.compile()
    return nc


_CACHE = {}


def _get_nc(reps, L_CH, H_CH, idx_cols, dr_cols, n_calls):
    key = (reps, L_CH, H_CH, idx_cols, dr_cols, n_calls)
    if key not in _CACHE:
        _CACHE[key] = _build(reps, L_CH, H_CH, idx_cols, dr_cols, n_calls)
    return _CACHE[key]


def run(x, edge_index, reps=1):
    ins, L_CH, H_CH, idx_cols, dr_cols, n_calls = _prep(x, edge_index)
    nc = _get_nc(reps, L_CH, H_CH, idx_cols, dr_cols, n_calls)
    res = run_bass_kernel_spmd(nc, ins, core_ids=list(range(N_CORES)))
    full = np.concatenate([res.results[c]["out"] for c in range(N_CORES)], axis=0)
    return full[:N_NODES]


def kernel(x, edge_index):
    return run(x, edge_index, reps=1)



# revision 3
# speedup vs baseline: 1.5510x; 1.5510x over previous
"""GNN message-passing (gather + segment_sum) Trainium2 Bass kernel.

Strategy (dst-sharded, on-chip gather + prefix-scan segment sum):
  - NeuronCore c owns dst nodes [c*6250, (c+1)*6250); no collective needed.
  - The full x table lives in SBUF as bf16 [128, 6251, 2]: partition 16g+q
    holds features (q, q+16) of src-bucket g's nodes (6250 nodes per bucket,
    plus a zero guard row at index 6250). Total 3.2MB, loaded once.
  - Edges are bucketed on host by (dst core, src bucket g, dst chunk of 512
    nodes) and sorted by dst inside each cell. Per chunk, one ap_gather
    (GpSimd ucode, SBUF->SBUF) pulls each stream's edge messages into
    msgs[16g+q, j, :] = x[src_j, (q, q+16)]; padded slots hit the guard row
    and read exact 0.
  - DVE tensor_tensor_scan computes an f32 inclusive prefix sum over each
    chunk's msgs (per feature parity, stride-2), carry-chained across chunks
    via the leading column of the scan tile.
  - A second ap_gather per chunk reads, for each of the 512 dst nodes, the
    prefix at its last edge (host-computed within-cell counts) into
    G[:, 1+n, :]; segment sum = G[1+n] - G[n] (G[0] = 0).
  - Final reduction per 128-node block: matmul lhsT=G[:, blk, c] (stride-2),
    rhs=R (R[p, j] = +/-1 iff p%16 == j) contracts over partitions, which
    simultaneously sums the 8 stream partials and transposes to [node, feat]
    in PSUM. Copy to SBUF, DMA out.

Self-contained: hardcodes the problem shapes from the spec.
"""

import numpy as np
import ml_dtypes

import concourse.bass as bass
import concourse.tile as tile
from concourse import bacc, mybir
from concourse.alu_op_type import AluOpType
from concourse.bass_utils import run_bass_kernel_spmd

BF16 = ml_dtypes.bfloat16

N_NODES = 50000
D_FEAT = 32
N_CORES = 8
N_STREAMS = 8  # gpsimd core groups per NeuronCore = src buckets
NPC = 6250  # dst nodes per NeuronCore
NPS = 6250  # src nodes per stream sub-table
GUARD = NPS  # gather index of the zero row
NCH = 512  # dst nodes per chunk
NCHUNKS = 13  # ceil(NPC / NCH)
NPAD = NCHUNKS * NCH  # 6656 padded dst nodes per core
BW = NCH // 16  # bidx columns per chunk


def _wrap16(a):
    """[..., n] -> [..., 16, n//16] wrapped: element j -> [j%16, j//16]."""
    n = a.shape[-1]
    return np.moveaxis(a.reshape(*a.shape[:-1], n // 16, 16), -1, -2)


def _prep(x, edge_index):
    x = np.asarray(x, np.float32)
    src = np.asarray(edge_index[0], np.int64)
    dst = np.asarray(edge_index[1], np.int64)
    E = src.size

    core = dst // NPC
    s = src // NPS
    dstl = dst - core * NPC
    srcl = (src - s * NPS).astype(np.int16)
    k = dstl // NCH

    ncells = N_CORES * N_STREAMS * NCHUNKS
    cell = (core * N_STREAMS + s) * NCHUNKS + k
    order = np.lexsort((dstl, cell))
    cell_s = cell[order]
    srcl_s = srcl[order]

    cnt = np.bincount(cell_s, minlength=ncells)
    # CH must be a multiple of 32: ap_gather ucode reads its (16-wrapped)
    # index list in uint32 units, so each chunk's idx-column slice must
    # start 4-byte aligned -> W = CH/16 must be even.
    CH = max(32, int(-(-int(cnt.max()) // 32) * 32))
    W = CH // 16

    first = np.cumsum(cnt) - cnt
    slot = np.arange(E) - first[cell_s]

    core_s = cell_s // (N_STREAMS * NCHUNKS)
    s_s = (cell_s // NCHUNKS) % N_STREAMS
    k_s = cell_s % NCHUNKS
    idx_arr = np.full((N_CORES, 128, NCHUNKS * W), GUARD, np.int16)
    idx_arr[core_s, 16 * s_s + slot % 16, k_s * W + slot // 16] = srcl_s

    # per-node inclusive within-cell edge counts -> boundary gather positions
    key = (core * N_STREAMS + s) * NPC + dstl
    nodecnt = np.bincount(key, minlength=N_CORES * N_STREAMS * NPC).reshape(
        N_CORES, N_STREAMS, NPC
    )
    nodecnt_pad = np.zeros((N_CORES, N_STREAMS, NPAD), np.int64)
    nodecnt_pad[:, :, :NPC] = nodecnt
    bpos = np.cumsum(
        nodecnt_pad.reshape(N_CORES, N_STREAMS, NCHUNKS, NCH), axis=3
    ).astype(np.int16)
    # [c, s, k, j] -> partition 16s + j%16, col k*BW + j//16
    bidx_arr = (
        _wrap16(bpos)  # [c, s, k, 16, BW]
        .transpose(0, 1, 3, 2, 4)
        .reshape(N_CORES, 128, NCHUNKS * BW)
    )

    # x table: partition 16g+q holds features (q, q+16) of bucket g's nodes
    xg = x.reshape(N_STREAMS, NPS, D_FEAT).transpose(0, 2, 1)  # [g, f, m]
    tab = np.zeros((N_STREAMS, 16, NPS + 1, 2), np.float32)
    tab[:, :, :NPS, 0] = xg[:, :16, :]
    tab[:, :, :NPS, 1] = xg[:, 16:, :]
    tab = tab.reshape(128, (NPS + 1) * 2).astype(BF16)

    rp = np.zeros((128, 16), np.float32)
    rp[np.arange(128), np.arange(128) % 16] = 1.0

    ins = []
    for c in range(N_CORES):
        ins.append(
            {
                "xtab": tab,
                "idx": idx_arr[c],
                "bidx": bidx_arr[c],
                "rpos": rp,
                "rneg": -rp,
            }
        )
    return ins, CH


def _build(reps, CH):
    W = CH // 16
    nc = bacc.Bacc(
        "TRN2", target_bir_lowering=False, debug=False, num_devices=N_CORES
    )
    f32 = mybir.dt.float32
    bf16 = mybir.dt.bfloat16
    i16 = mybir.dt.int16
    xtab_d = nc.dram_tensor("xtab", [128, (NPS + 1) * 2], bf16, kind="ExternalInput")
    idx_d = nc.dram_tensor("idx", [128, NCHUNKS * W], i16, kind="ExternalInput")
    bidx_d = nc.dram_tensor("bidx", [128, NCHUNKS * BW], i16, kind="ExternalInput")
    rpos_d = nc.dram_tensor("rpos", [128, 16], f32, kind="ExternalInput")
    rneg_d = nc.dram_tensor("rneg", [128, 16], f32, kind="ExternalInput")
    out_d = nc.dram_tensor("out", [NPAD, D_FEAT], f32, kind="ExternalOutput")

    with tile.TileContext(nc) as tc:
        with (
            tc.tile_pool(name="meta", bufs=1) as meta,
            tc.tile_pool(name="msg", bufs=2) as msgp,
            tc.tile_pool(name="scan", bufs=2) as scanp,
            tc.tile_pool(name="gbuf", bufs=1) as gp,
            tc.tile_pool(name="ps", bufs=2, space="PSUM") as psp,
            tc.tile_pool(name="stage", bufs=2) as stagep,
        ):
            xtab_t = meta.tile([128, (NPS + 1) * 2], bf16)
            nc.sync.dma_start(xtab_t[:], xtab_d.ap())
            idx_t = meta.tile([128, NCHUNKS * W], i16)
            nc.sync.dma_start(idx_t[:], idx_d.ap())
            bidx_t = meta.tile([128, NCHUNKS * BW], i16)
            nc.sync.dma_start(bidx_t[:], bidx_d.ap())
            rpos_t = meta.tile([128, 16], f32)
            nc.sync.dma_start(rpos_t[:], rpos_d.ap())
            rneg_t = meta.tile([128, 16], f32)
            nc.sync.dma_start(rneg_t[:], rneg_d.ap())

            def body(_=None):
                G = gp.tile([128, NPAD + 1, 2], f32)
                nc.vector.memset(G[:, 0:1, :], 0.0)
                msgs = [None] * NCHUNKS
                S = [None] * NCHUNKS

                def gather(kk):
                    m = msgp.tile([128, CH, 2], bf16)
                    nc.gpsimd.ap_gather(
                        m[:], xtab_t[:], idx_t[:, kk * W : (kk + 1) * W],
                        128, NPS + 1, 2, CH,
                    )
                    msgs[kk] = m

                gather(0)
                for k in range(NCHUNKS):
                    if k + 1 < NCHUNKS:
                        gather(k + 1)
                    Sk = scanp.tile([128, CH + 1, 2], f32)
                    if k == 0:
                        nc.vector.memset(Sk[:, 0:1, :], 0.0)
                    else:
                        nc.scalar.copy(Sk[:, 0:1, :], S[k - 1][:, CH : CH + 1, :])
                    for c in (0, 1):
                        nc.vector.tensor_tensor_scan(
                            out=Sk[:, 1 : CH + 1, c],
                            data0=msgs[k][:, :, c],
                            data1=msgs[k][:, :, c],
                            initial=Sk[:, 0:1, c],
                            op0=AluOpType.add,
                            op1=AluOpType.bypass,
                        )
                    nc.gpsimd.ap_gather(
                        G[:, 1 + k * NCH : 1 + (k + 1) * NCH, :],
                        Sk[:],
                        bidx_t[:, k * BW : (k + 1) * BW],
                        128, CH + 1, 2, NCH,
                    )
                    S[k] = Sk

                for b in range(NPAD // 128):
                    base = b * 128
                    ps = psp.tile([128, D_FEAT], f32)
                    nc.tensor.matmul(
                        out=ps[:, 0:16], lhsT=G[:, 1 + base : 1 + base + 128, 0],
                        rhs=rpos_t[:], start=True, stop=False,
                    )
                    nc.tensor.matmul(
                        out=ps[:, 0:16], lhsT=G[:, base : base + 128, 0],
                        rhs=rneg_t[:], start=False, stop=True,
                    )
                    nc.tensor.matmul(
                        out=ps[:, 16:32], lhsT=G[:, 1 + base : 1 + base + 128, 1],
                        rhs=rpos_t[:], start=True, stop=False,
                    )
                    nc.tensor.matmul(
                        out=ps[:, 16:32], lhsT=G[:, base : base + 128, 1],
                        rhs=rneg_t[:], start=False, stop=True,
                    )
                    st = stagep.tile([128, D_FEAT], f32)
                    nc.scalar.copy(st[:], ps[:])
                    nc.sync.dma_start(out_d.ap()[base : base + 128, :], st[:])

            if reps == 1:
                body()
            else:
                with tc.For_i(0, reps) as _i:
                    body(_i)
    nc.compile()
    return nc


_CACHE = {}


def _get_nc(reps, CH):
    key = (reps, CH)
    if key not in _CACHE:
        _CACHE[key] = _build(reps, CH)
    return _CACHE[key]


def run(x, edge_index, reps=1):
    ins, CH = _prep(x, edge_index)
    nc = _get_nc(reps, CH)
    res = run_bass_kernel_spmd(nc, ins, core_ids=list(range(N_CORES)))
    full = np.concatenate(
        [res.results[c]["out"][:NPC] for c in range(N_CORES)], axis=0
    )
    return full


def kernel(x, edge_index):
    return run(x, edge_index, reps=1)


# revision 14
# speedup vs baseline: 8.5240x; 5.4958x over previous
"""GNN message-passing (gather + segment_sum) Trainium2 Bass kernel.

Strategy (dst-sharded, on-chip gather + prefix-scan segment sum):
  - NeuronCore c owns dst nodes [c*6250, (c+1)*6250); no collective needed.
  - The full x table lives in SBUF as bf16 [128, 6251, 2]: partition 16g+q
    holds features (q, q+16) of src-bucket g's nodes (6250 nodes per bucket,
    plus a zero guard row at index 6250). Total 3.2MB, loaded once.
  - Edges are bucketed on host by (dst core, src bucket g, dst chunk of 512
    nodes) and sorted by dst inside each cell. Per chunk, one ap_gather
    (GpSimd ucode, SBUF->SBUF) pulls each stream's edge messages into
    msgs[16g+q, j, :] = x[src_j, (q, q+16)]; padded slots hit the guard row
    and read exact 0. Chunk idx counts CH_k are per-chunk (multiples of 32:
    the ucode reads the 16-wrapped idx list in uint32 units, so every
    chunk's idx-column slice must start 4-byte aligned).
  - DVE tensor_tensor_scan computes an f32 inclusive prefix sum over each
    chunk's msgs (per feature parity, stride-2), carry-chained across chunks
    via the leading column of the scan tile.
  - A second ap_gather per chunk reads, for each of the 512 dst nodes, the
    prefix at its last edge (host-computed within-cell counts) into
    G[:, 1+n, :]; segment sum = G[1+n] - G[n] (G[0] = 0).
  - Final reduction, interleaved into the chunk loop (4 blocks of 128 nodes
    per chunk): matmul lhsT=G[:, blk, c] (stride-2), rhs=R (R[p, j] = +/-1
    iff p%16 == j) contracts over partitions, which simultaneously sums the
    8 stream partials and transposes to [node, feat] in PSUM. Copy to SBUF,
    DMA out.

Self-contained: hardcodes the problem shapes from the spec.
"""

import numpy as np
import ml_dtypes

import concourse.bass as bass
import concourse.tile as tile
from concourse import bacc, mybir
from concourse.alu_op_type import AluOpType
from concourse.bass_utils import run_bass_kernel_spmd

BF16 = ml_dtypes.bfloat16

N_NODES = 50000
D_FEAT = 32
N_CORES = 8
N_STREAMS = 8  # gpsimd core groups per NeuronCore = src buckets
NPC = 6250  # dst nodes per NeuronCore
NPS = 6250  # src nodes per stream sub-table
GUARD = NPS  # gather index of the zero row
NCH = 512  # dst nodes per chunk
NCHUNKS = 13  # ceil(NPC / NCH)
NPAD = NCHUNKS * NCH  # 6656 padded dst nodes per core
BW = NCH // 16  # bidx columns per chunk
BLKS_PER_CHUNK = NCH // 128  # 4


def _wrap16(a):
    """[..., n] -> [..., 16, n//16] wrapped: element j -> [j%16, j//16]."""
    n = a.shape[-1]
    return np.moveaxis(a.reshape(*a.shape[:-1], n // 16, 16), -1, -2)


def _prep(x, edge_index):
    x = np.asarray(x, np.float32)
    src = np.asarray(edge_index[0], np.int64)
    dst = np.asarray(edge_index[1], np.int64)
    E = src.size

    core = dst // NPC
    s = src // NPS
    dstl = dst - core * NPC
    srcl = (src - s * NPS).astype(np.int16)
    k = dstl // NCH

    ncells = N_CORES * N_STREAMS * NCHUNKS
    cell = (core * N_STREAMS + s) * NCHUNKS + k
    order = np.lexsort((dstl, cell))
    cell_s = cell[order]
    srcl_s = srcl[order]

    cnt = np.bincount(cell_s, minlength=ncells)
    # per-chunk idx count: multiple of 32 so every chunk's idx slice starts
    # uint32-aligned in the 16-wrapped layout
    chmax = cnt.reshape(N_CORES * N_STREAMS, NCHUNKS).max(axis=0)
    CHs = tuple(int(v) for v in np.maximum(32, ((chmax + 31) // 32) * 32))
    Ws = [c // 16 for c in CHs]
    wofs = np.concatenate([[0], np.cumsum(Ws)])

    first = np.cumsum(cnt) - cnt
    slot = np.arange(E) - first[cell_s]

    core_s = cell_s // (N_STREAMS * NCHUNKS)
    s_s = (cell_s // NCHUNKS) % N_STREAMS
    k_s = cell_s % NCHUNKS
    idx_cols = int(wofs[-1])
    idx_arr = np.full((N_CORES, 128, idx_cols), GUARD, np.int16)
    idx_arr[core_s, 16 * s_s + slot % 16, wofs[k_s] + slot // 16] = srcl_s

    # per-node inclusive within-cell edge counts -> boundary gather positions
    key = (core * N_STREAMS + s) * NPC + dstl
    nodecnt = np.bincount(key, minlength=N_CORES * N_STREAMS * NPC).reshape(
        N_CORES, N_STREAMS, NPC
    )
    nodecnt_pad = np.zeros((N_CORES, N_STREAMS, NPAD), np.int64)
    nodecnt_pad[:, :, :NPC] = nodecnt
    bpos = np.cumsum(
        nodecnt_pad.reshape(N_CORES, N_STREAMS, NCHUNKS, NCH), axis=3
    ).astype(np.int16)
    # [c, s, k, j] -> partition 16s + j%16, col k*BW + j//16
    bidx_arr = (
        _wrap16(bpos)  # [c, s, k, 16, BW]
        .transpose(0, 1, 3, 2, 4)
        .reshape(N_CORES, 128, NCHUNKS * BW)
    )

    # x table: partition 16g+q holds features (q, q+16) of bucket g's nodes
    xg = x.reshape(N_STREAMS, NPS, D_FEAT).transpose(0, 2, 1)  # [g, f, m]
    tab = np.zeros((N_STREAMS, 16, NPS + 1, 2), np.float32)
    tab[:, :, :NPS, 0] = xg[:, :16, :]
    tab[:, :, :NPS, 1] = xg[:, 16:, :]
    tab = tab.reshape(128, NPS + 1, 2).astype(BF16)

    rp = np.zeros((128, 16), np.float32)
    rp[np.arange(128), np.arange(128) % 16] = 1.0

    ins = []
    for c in range(N_CORES):
        ins.append(
            {
                "xtab": tab,
                "idx": idx_arr[c],
                "bidx": bidx_arr[c],
                "rpos": rp,
                "rneg": -rp,
            }
        )
    return ins, CHs


def _build(reps, CHs):
    Ws = [c // 16 for c in CHs]
    idx_cols = sum(Ws)
    CHM = max(CHs)
    nc = bacc.Bacc(
        "TRN2", target_bir_lowering=False, debug=False, num_devices=N_CORES
    )
    f32 = mybir.dt.float32
    bf16 = mybir.dt.bfloat16
    i16 = mybir.dt.int16
    xtab_d = nc.dram_tensor("xtab", [128, NPS + 1, 2], bf16, kind="ExternalInput")
    idx_d = nc.dram_tensor("idx", [128, idx_cols], i16, kind="ExternalInput")
    bidx_d = nc.dram_tensor("bidx", [128, NCHUNKS * BW], i16, kind="ExternalInput")
    rpos_d = nc.dram_tensor("rpos", [128, 16], f32, kind="ExternalInput")
    rneg_d = nc.dram_tensor("rneg", [128, 16], f32, kind="ExternalInput")
    out_d = nc.dram_tensor("out", [NPAD, D_FEAT], f32, kind="ExternalOutput")

    with tile.TileContext(nc) as tc:
        with (
            tc.tile_pool(name="meta", bufs=1) as meta,
            tc.tile_pool(name="msg", bufs=3) as msgp,
            tc.tile_pool(name="scan", bufs=2) as scanp,
            tc.tile_pool(name="gbuf", bufs=2) as gp,
            tc.tile_pool(name="ps", bufs=4, space="PSUM") as psp,
            tc.tile_pool(name="stage", bufs=4) as stagep,
        ):
            xtab_t = meta.tile([128, NPS + 1, 2], bf16)
            nc.sync.dma_start(xtab_t[:], xtab_d.ap())
            idx_t = meta.tile([128, idx_cols], i16)
            nc.sync.dma_start(idx_t[:], idx_d.ap())
            bidx_t = meta.tile([128, NCHUNKS * BW], i16)
            nc.sync.dma_start(bidx_t[:], bidx_d.ap())
            rpos_t = meta.tile([128, 16], f32)
            nc.sync.dma_start(rpos_t[:], rpos_d.ap())
            rneg_t = meta.tile([128, 16], f32)
            nc.sync.dma_start(rneg_t[:], rneg_d.ap())

            def body(_=None):
                G = gp.tile([128, NPAD + 1, 2], f32)
                nc.vector.memset(G[:, 0:1, :], 0.0)
                msgs = [None] * NCHUNKS
                S = [None] * NCHUNKS

                def gather(kk):
                    ch = CHs[kk]
                    m = msgp.tile([128, CHM, 2], bf16)
                    nc.gpsimd.ap_gather(
                        m[:, 0:ch, :],
                        xtab_t[:],
                        idx_t[:, wofs_k[kk] : wofs_k[kk] + Ws[kk]],
                        128, NPS + 1, 2, ch,
                    )
                    msgs[kk] = m

                wofs_k = [0]
                for w in Ws:
                    wofs_k.append(wofs_k[-1] + w)

                gather(0)
                for k in range(NCHUNKS):
                    ch = CHs[k]
                    if k + 1 < NCHUNKS:
                        gather(k + 1)
                    Sk = scanp.tile([128, CHM + 1, 2], f32)
                    if k == 0:
                        nc.vector.memset(Sk[:, 0:1, :], 0.0)
                    else:
                        # carry copy on DVE: keeps the scan chain entirely on
                        # the vector engine (the scalar engine's program order
                        # would serialize it behind stage copies -> matmuls ->
                        # boundary gathers)
                        pch = CHs[k - 1]
                        nc.vector.tensor_scalar_add(
                            Sk[:, 0:1, :], S[k - 1][:, pch : pch + 1, :], 0.0
                        )
                    for c in (0, 1):
                        nc.vector.tensor_tensor_scan(
                            out=Sk[:, 1 : ch + 1, c],
                            data0=msgs[k][:, 0:ch, c],
                            data1=msgs[k][:, 0:ch, c],
                            initial=Sk[:, 0:1, c],
                            op0=AluOpType.add,
                            op1=AluOpType.bypass,
                        )
                    # last chunk: only nodes 6144..6271 matter (<= 6249 real);
                    # gather 128 boundaries instead of 512 and emit 1 block
                    nb = NCH if k < NCHUNKS - 1 else 128
                    nc.gpsimd.ap_gather(
                        G[:, 1 + k * NCH : 1 + k * NCH + nb, :],
                        Sk[:, 0 : ch + 1, :],
                        bidx_t[:, k * BW : k * BW + nb // 16],
                        128, ch + 1, 2, nb,
                    )
                    S[k] = Sk

                    blk_end = min(BLKS_PER_CHUNK * (k + 1), (NPC + 127) // 128)
                    for b in range(BLKS_PER_CHUNK * k, blk_end):
                        base = b * 128
                        ps = psp.tile([128, D_FEAT], f32)
                        nc.tensor.matmul(
                            out=ps[:, 0:16],
                            lhsT=G[:, 1 + base : 1 + base + 128, 0],
                            rhs=rpos_t[:], start=True, stop=False,
                        )
                        nc.tensor.matmul(
                            out=ps[:, 0:16],
                            lhsT=G[:, base : base + 128, 0],
                            rhs=rneg_t[:], start=False, stop=True,
                        )
                        nc.tensor.matmul(
                            out=ps[:, 16:32],
                            lhsT=G[:, 1 + base : 1 + base + 128, 1],
                            rhs=rpos_t[:], start=True, stop=False,
                        )
                        nc.tensor.matmul(
                            out=ps[:, 16:32],
                            lhsT=G[:, base : base + 128, 1],
                            rhs=rneg_t[:], start=False, stop=True,
                        )
                        st = stagep.tile([128, D_FEAT], f32)
                        nc.scalar.copy(st[:], ps[:])
                        nc.sync.dma_start(out_d.ap()[base : base + 128, :], st[:])

            if reps == 1:
                body()
            else:
                with tc.For_i(0, reps) as _i:
                    body(_i)
    nc.compile()
    return nc


_CACHE = {}


def _get_nc(reps, CHs):
    key = (reps, CHs)
    if key not in _CACHE:
        _CACHE[key] = _build(reps, CHs)
    return _CACHE[key]


def run(x, edge_index, reps=1):
    ins, CHs = _prep(x, edge_index)
    nc = _get_nc(reps, CHs)
    res = run_bass_kernel_spmd(nc, ins, core_ids=list(range(N_CORES)))
    full = np.concatenate(
        [res.results[c]["out"][:NPC] for c in range(N_CORES)], axis=0
    )
    return full


def kernel(x, edge_index):
    return run(x, edge_index, reps=1)
